# revision 42
# baseline (speedup 1.0000x reference)
"""Trainium2 Bass kernel for nn_AttentionBlock (64, 512, 16) / three 8192x8192 Linears.

Strategy (8 NeuronCores, single NEFF, one launch):
  Projections (column-sharded, fp8 DoubleRow): each core c owns output
    columns [1024c, 1024(c+1)) of each Linear (= w positions [64c,
    64(c+1)), all 16 d). Weights are pre-transposed, pre-scaled by 64
    (keeps N(0, 0.02^2) values out of e4m3 subnormals) and pre-swizzled
    on host so every 1MB weight DMA is contiguous with 8KB rows. x is
    also cast to e4m3.
  Phase order: V streams FIRST, then Q,K. The v AllToAll is therefore
    the first collective and absorbs mesh init + core launch skew under
    the q/k streaming; the q/k AllToAll then pays only transfer time,
    and its staging lands on idle DMA rings right after streaming ends.
  AllToAll: core c sends batch-block j of its v (then q/k) shard to
    core j, fp8 payloads. After both, each core holds full q/k/v for
    its own 8 batches -- the attention program is fully static per core.
  Attention (batch-sharded, 8 batches/core): alphas are built
    transposed [k, q] so softmax over the *query* axis is a free-dim
    reduction (exp via ScalarE with fused accum row-sum; the 64^2/8^2
    weight/payload scales fold into the exp input scale). The softmax
    reciprocal folds into the exp'd alphas (x256 for fp8 range); the
    second einsum runs as fp8 DoubleRow matmuls with the two kw blocks
    of a pair as the reduction k-tiles; per-batch sigmoids (scale 1/256
    folds the alpha prescale away, and the 1/64 v scale was folded into
    the v cast) write 32-aligned quadrants of packed output tiles, so
    the tail is two residual adds and eight 32KB output DMAs.
  Host: gathers per-core (128, 512) outputs, transposes back.
"""

import math

import numpy as np
import ml_dtypes

import concourse.bass as bass
import concourse.bacc as bacc
import concourse.mybir as mybir
import concourse.tile as tile
import concourse.bass_utils as bass_utils

N_CORES = 8
BS, W_DIM, D = 64, 512, 16
K = W_DIM * D            # 8192 contraction dim
CPC = K // N_CORES       # 1024 output cols per core
WPC = W_DIM // N_CORES   # 64 w positions per core
BPC = BS // N_CORES      # 8 batches per core
NKT = K // 128           # 64 k-tiles
CHUNK = 8                # k-tiles per weight DMA (4 DoubleRow pairs)
NCH = NKT // CHUNK       # 8 weight chunks
WSCALE = 64.0            # host-side weight pre-scale (fp8 subnormal dodge)
QKSCALE = 8.0            # q/k payload post-scale divisor (fp8 range fit)
EASCALE = 256.0          # exp'd-alpha pre-scale (fp8 subnormal dodge)
EXP_SCALE = QKSCALE * QKSCALE / (math.sqrt(K) * WSCALE * WSCALE)

_CACHE: dict = {}


def _build(wbufs: int = 5):
    f8 = mybir.dt.float8e4
    bf16 = mybir.dt.bfloat16
    f32 = mybir.dt.float32
    DR = mybir.MatmulPerfMode.DoubleRow

    nc = bacc.Bacc("TRN2", target_bir_lowering=False, debug=False,
                   num_devices=N_CORES)

    # xt is host-preswizzled to [128 p, 64 kt, 64 b] so the load is one
    # fully contiguous DMA.
    xt_d = nc.dram_tensor("xt", [128, NKT * BS], f8, kind="ExternalInput")
    # weights host-swizzled to [m, p, j, c] -> each 128-row slice is one
    # contiguous 1MB chunk whose rows are 8KB.
    w_d = [nc.dram_tensor(n, [NCH * 128, CHUNK * CPC], f8,
                          kind="ExternalInput")
           for n in ("wq", "wk", "wv")]
    b_d = [nc.dram_tensor(n, [1, CPC], bf16, kind="ExternalInput")
           for n in ("bq", "bk", "bv")]
    # residual x^T, padded to 32-partition quadrants (4 batches per tile,
    # batch q at partitions [32q, 32q+16))
    xtp_d = nc.dram_tensor("xtp", [2 * 128, W_DIM], f32, kind="ExternalInput")
    out_d = nc.dram_tensor("out", [BPC * D, W_DIM], f32, kind="ExternalOutput")

    hwdge = [nc.sync, nc.scalar]          # the two independent HWDGE rings

    with tile.TileContext(nc) as tc:
        with (
            tc.tile_pool(name="constp", bufs=1) as constp,
            tc.tile_pool(name="sbp", bufs=1) as sbp,
            tc.tile_pool(name="dramp", bufs=1, space="DRAM") as dramp,
            tc.tile_pool(name="wpa", bufs=wbufs) as wpa,
            tc.tile_pool(name="wpb", bufs=wbufs) as wpb,
        ):
            xt_sb = constp.tile([128, NKT, BS], f8)
            nc.sync.dma_start(
                xt_sb[:], xt_d[:, :].rearrange("p (kt b) -> p kt b", kt=NKT))
            xtp_sb = constp.tile([128, 2, W_DIM], f32)
            nc.scalar.dma_start(
                xtp_sb[:], xtp_d[:, :].rearrange("(g p) w -> p g w", g=2))
            ones = constp.tile([1, BS], bf16)
            nc.gpsimd.memset(ones[:], 1.0)
            # warm the ScalarE Exp table off the critical path (the
            # table is single-slot, so only one function can be warmed)
            tbl = constp.tile([1, 8], f32)
            nc.scalar.activation(tbl[:], ones[:, 0:8],
                                 mybir.ActivationFunctionType.Exp)
            b_sb = []
            for t in range(3):
                bt = constp.tile([1, CPC], bf16, name=f"bias{t}")
                nc.scalar.dma_start(bt[:], b_d[t][:, :])
                b_sb.append(bt)

            qk_sb = sbp.tile([BS, 2 * CPC], f8, name="qk_sb", tag="qk_sb")
            v_sb = sbp.tile([BS, CPC], f8, name="v_sb", tag="v_sb")
            a2a1_in = dramp.tile([N_CORES, 2, BPC, CPC], f8,
                                 tag="a2a1_in", name="a2a1_in")
            a2a1_out = dramp.tile([N_CORES, 2, BPC, CPC], f8,
                                  tag="a2a1_out", name="a2a1_out")
            a2a2_in = dramp.tile([N_CORES, BPC, CPC], f8,
                                 tag="a2a2_in", name="a2a2_in")
            a2a2_out = dramp.tile([N_CORES, BPC, CPC], f8,
                                  tag="a2a2_out", name="a2a2_out")

            def w_chunk_matmuls(m, psum, wt):
                for jj in range(0, CHUNK, 2):
                    for h in range(2):
                        nc.tensor.matmul(
                            psum[:, h * 512:(h + 1) * 512],
                            xt_sb[:, CHUNK * m + jj:CHUNK * m + jj + 2, :],
                            wt[:, jj:jj + 2, h * 512:(h + 1) * 512],
                            start=(m == 0 and jj == 0), stop=False,
                            perf_mode=DR)

            def bias_matmuls(t, psum):
                for h in range(2):
                    nc.tensor.matmul(
                        psum[:, h * 512:(h + 1) * 512],
                        ones[:],
                        b_sb[t][:, h * 512:(h + 1) * 512],
                        start=False, stop=True)

            # ---- phase QK: q,k projection ----
            with tc.tile_pool(name="qkps", bufs=1, space="PSUM") as qkps:
                psA = [qkps.tile([BS, CPC], f32, name=f"ps{t}")
                       for t in range(2)]
                for m in range(NCH):
                    for t in range(2):
                        wt = wpa.tile([128, CHUNK, CPC], f8,
                                      tag=f"w{t}", name=f"wt{t}")
                        hwdge[(m * 2 + t) % 2].dma_start(
                            wt[:],
                            w_d[t][128 * m:128 * (m + 1), :].rearrange(
                                "p (j c) -> p j c", j=CHUNK))
                        w_chunk_matmuls(m, psA[t], wt)
                for t in range(2):
                    bias_matmuls(t, psA[t])
                    # permute cols (w d) -> (d w) while leaving PSUM;
                    # scale into fp8 payload range
                    nc.vector.tensor_scalar_mul(
                        qk_sb[:, t * CPC:(t + 1) * CPC].rearrange(
                            "b (d w) -> b d w", w=WPC),
                        psA[t].rearrange("b (w d) -> b d w", d=D),
                        1.0 / QKSCALE)

            for t in range(2):
                nc.gpsimd.dma_start(
                    a2a1_in[:, t, :, :],
                    qk_sb[:, t * CPC:(t + 1) * CPC])
            # hold v streaming back until the staging data lands so the
            # A2A1 trigger isn't starved by DMA-engine congestion
            tc.strict_bb_all_engine_barrier()
            nc.gpsimd.collective_compute(
                "AllToAll", mybir.AluOpType.bypass,
                replica_groups=[list(range(N_CORES))],
                ins=[a2a1_in.opt()], outs=[a2a1_out.opt()])

            # ---- attention pools open early so the q/k gathers sit
            # ahead of the v staging + A2A2 trigger in the gpsimd FIFO ----
            with (
                tc.tile_pool(name="attps", bufs=1, space="PSUM") as attps,
                tc.tile_pool(name="attp", bufs=3) as attp,
                tc.tile_pool(name="keepp", bufs=1) as keepp,
            ):
                # dens + q/k gathers up-front: memsets run under the
                # collective, gathers fire the moment it completes
                qkT_tiles = []
                den_tiles = []
                for b in range(BPC):
                    den = keepp.tile([128, 4], f32, tag=f"den{b}",
                                     name=f"den{b}")
                    nc.gpsimd.memset(den[:], 0.0)
                    den_tiles.append(den)
                for b in range(BPC):
                    qkT = keepp.tile([D, 2, N_CORES, WPC], f8,
                                     tag=f"qkT{b}", name=f"qkT{b}")
                    for t in range(2):
                        nc.gpsimd.dma_start(
                            qkT[:, t, :, :],
                            a2a1_out[:, t, b, :].rearrange(
                                "i (d w) -> d i w", d=D))
                    qkT_tiles.append(qkT)

                # ---- phase V: v projection, overlapped with the q/k
                # AllToAll and attention part A ----
                with tc.tile_pool(name="vps", bufs=1, space="PSUM") as vps:
                    psV = vps.tile([BS, CPC], f32, name="psv")
                    for m in range(NCH):
                        wt = wpb.tile([128, CHUNK, CPC], f8, tag="w2",
                                      name="wt2")
                        hwdge[m % 2].dma_start(
                            wt[:],
                            w_d[2][128 * m:128 * (m + 1), :].rearrange(
                                "p (j c) -> p j c", j=CHUNK))
                        w_chunk_matmuls(m, psV, wt)
                    bias_matmuls(2, psV)
                    # fold the 1/64 weight prescale into the fp8 v payload
                    nc.vector.tensor_scalar_mul(v_sb[:], psV[:],
                                                1.0 / WSCALE)

                nc.gpsimd.dma_start(
                    a2a2_in[:, :, :].rearrange("j b c -> j b c"),
                    v_sb[:, :])
                nc.gpsimd.collective_compute(
                    "AllToAll", mybir.AluOpType.bypass,
                    replica_groups=[list(range(N_CORES))],
                    ins=[a2a2_in.opt()], outs=[a2a2_out.opt()])
                a2a_v = a2a2_out.rearrange("(kw h) b c -> kw h b c", h=2)

                # v tiles for all batches: resident long before part B
                vt_tiles = []
                for b in range(BPC):
                    vt = keepp.tile([128, 4, D], f8, tag=f"vt{b}",
                                    name=f"vt{b}")
                    for half in range(2):
                        hwdge[half].dma_start(
                            vt[64 * half:64 * half + 64, :, :],
                            a2a_v[:, half, b, :].rearrange(
                                "i (w d) -> w i d", d=D))
                    vt_tiles.append(vt)

                sg_tiles = [keepp.tile([128, W_DIM], f32, tag=f"sg{g}",
                                       name=f"sg{g}") for g in range(2)]
                eas_tiles = {}
                for b in range(BPC):
                    qkT = qkT_tiles[b]
                    den = den_tiles[b]
                    ea_tiles = []
                    for j in range(2):
                        aT2 = attps.tile([128, 2, 512], f32, tag="aT2",
                                         name="aT2", bufs=2)
                        for h in range(2):
                            kw = 2 * j + h
                            nc.tensor.matmul(
                                aT2[:, h, :], qkT[:, 1, 2 * kw:2 * kw + 2, :],
                                qkT[:, 0, :, :], start=True, stop=True)
                            ea = attp.tile([128, 512], bf16, tag=f"ea{kw}",
                                           name="ea")
                            nc.scalar.activation(
                                ea[:], aT2[:, h, :],
                                mybir.ActivationFunctionType.Exp,
                                scale=EXP_SCALE, accum_out=den[:, kw:kw + 1])
                            ea_tiles.append(ea)
                    rec = attp.tile([128, 4], f32, tag="rec", name="rec")
                    nc.vector.reciprocal(rec[:], den[:])
                    eas_pair = []
                    for j in range(2):
                        # fold softmax denominator; x256 keeps the near-
                        # uniform softmax weights in fp8 normal range
                        eas = keepp.tile([128, 2, 512], f8,
                                         tag=f"eas{b}_{j}",
                                         name=f"eas{b}_{j}")
                        for h in range(2):
                            kw = 2 * j + h
                            nc.vector.tensor_scalar(
                                eas[:, h, :], ea_tiles[kw][:],
                                rec[:, kw:kw + 1], EASCALE,
                                op0=mybir.AluOpType.mult,
                                op1=mybir.AluOpType.mult)
                        eas_pair.append(eas)
                    eas_tiles[b] = eas_pair
                # dummy sigmoid: pulls the single-slot ACT table swap
                # into the scalar-idle window while rT matmuls run,
                # instead of delaying the first real sigmoid
                sgw = attp.tile([1, 8], f32, tag="sgw", name="sgw")
                nc.scalar.activation(sgw[:], ones[:, 0:8],
                                     mybir.ActivationFunctionType.Sigmoid)
                # part B as a separate loop: keeps the in-order scalar
                # queue as [all exps][all sigmoids] with no mid-stalls
                for b in range(BPC):
                    rT = attps.tile([D, W_DIM], f32, tag="rT", name="rT",
                                    bufs=2)
                    for j in range(2):
                        # fp8 DoubleRow: the two kw blocks of a pair ride
                        # as the two reduction k-tiles
                        nc.tensor.matmul(
                            rT[:], vt_tiles[b][:, 2 * j:2 * j + 2, :],
                            eas_tiles[b][j][:],
                            start=(j == 0), stop=(j == 1),
                            perf_mode=DR)
                    # per-batch sigmoid straight from PSUM into the
                    # 32-aligned quadrant of the packed output tile
                    nc.scalar.activation(
                        sg_tiles[b // 4][32 * (b % 4):32 * (b % 4) + D, :],
                        rT[:], mybir.ActivationFunctionType.Sigmoid,
                        scale=1.0 / EASCALE)
                for g in range(2):
                    oo = keepp.tile([128, W_DIM], f32, tag=f"oo{g}",
                                    name=f"oo{g}")
                    nc.vector.tensor_add(oo[:], sg_tiles[g][:],
                                         xtp_sb[:, g, :])
                    for b4 in range(4):
                        b = 4 * g + b4
                        hwdge[b % 2].dma_start(
                            out_d[D * b:D * (b + 1), :],
                            oo[32 * b4:32 * b4 + D, :])

    nc.compile()
    return nc


def _prep_in_maps(x_in, Wq, bq, Wk, bk, Wv, bv):
    f8 = ml_dtypes.float8_e4m3
    bf16 = ml_dtypes.bfloat16
    x_flat = np.ascontiguousarray(np.asarray(x_in, np.float32).reshape(BS, K))
    # swizzled x^T: [128 p, kt, b] contiguous
    xt = np.ascontiguousarray(
        x_flat.T.reshape(NKT, 128, BS).transpose(1, 0, 2)
    ).reshape(128, NKT * BS).astype(f8)
    # W^T scaled by 64, swizzled to [m, p, j, c] per core slice
    ws = [np.ascontiguousarray(np.asarray(W, np.float32).T) * WSCALE
          for W in (Wq, Wk, Wv)]
    bs = [(np.asarray(b, np.float32) * WSCALE).reshape(1, K).astype(bf16)
          for b in (bq, bk, bv)]
    xtp = np.ascontiguousarray(
        np.asarray(x_in, np.float32).transpose(0, 2, 1))       # (BS, D, W)

    in_maps = []
    for c in range(N_CORES):
        cs = slice(CPC * c, CPC * (c + 1))
        m = {"xt": xt}
        for nm, w in zip(("wq", "wk", "wv"), ws):
            m[nm] = np.ascontiguousarray(
                w[:, cs].reshape(NCH, CHUNK, 128, CPC).transpose(0, 2, 1, 3)
            ).reshape(NCH * 128, CHUNK * CPC).astype(f8)
        for nm, b in zip(("bq", "bk", "bv"), bs):
            m[nm] = np.ascontiguousarray(b[:, cs])
        xp = np.zeros((2, 4, 32, W_DIM), np.float32)
        xp[:, :, :D, :] = xtp[BPC * c:BPC * (c + 1)].reshape(2, 4, D, W_DIM)
        m["xtp"] = xp.reshape(2 * 128, W_DIM)
        in_maps.append(m)
    return in_maps


def _assemble(results):
    out = np.empty((BS, W_DIM, D), np.float32)
    for c in range(N_CORES):
        o = results[c]["out"].reshape(BPC, D, W_DIM)
        out[BPC * c:BPC * (c + 1)] = o.transpose(0, 2, 1)
    return out


def get_nc():
    if "nc" not in _CACHE:
        _CACHE["nc"] = _build()
    return _CACHE["nc"]


def kernel(x_in, Wq, bq, Wk, bk, Wv, bv):
    nc = get_nc()
    in_maps = _prep_in_maps(x_in, Wq, bq, Wk, bk, Wv, bv)
    res = bass_utils.run_bass_kernel_spmd(
        nc, in_maps, core_ids=list(range(N_CORES)))
    return _assemble(res.results)


# revision 43
# speedup vs baseline: 1.4544x; 1.4544x over previous
"""Trainium2 Bass kernel for nn_AttentionBlock (64, 512, 16) / three 8192x8192 Linears.

Strategy (8 NeuronCores, single NEFF, one launch):
  Projections (column-sharded, fp8 DoubleRow): each core c owns output
    columns [1024c, 1024(c+1)) of each Linear (= w positions [64c,
    64(c+1)), all 16 d). Weights are pre-transposed, pre-scaled by 64
    (keeps N(0, 0.02^2) values out of e4m3 subnormals) and pre-swizzled
    on host so every 1MB weight DMA is contiguous with 8KB rows. x is
    also cast to e4m3.
  Phase order: V streams FIRST, then Q,K. The v AllToAll is therefore
    the first collective and absorbs mesh init + core launch skew under
    the q/k streaming; the q/k AllToAll then pays only transfer time,
    and its staging lands on idle DMA rings right after streaming ends.
  AllToAll: core c sends batch-block j of its v (then q/k) shard to
    core j, fp8 payloads. After both, each core holds full q/k/v for
    its own 8 batches -- the attention program is fully static per core.
  Attention (batch-sharded, 8 batches/core): alphas are built
    transposed [k, q] so softmax over the *query* axis is a free-dim
    reduction (exp via ScalarE with fused accum row-sum; the 64^2/8^2
    weight/payload scales fold into the exp input scale). The softmax
    reciprocal folds into the exp'd alphas (x256 for fp8 range); the
    second einsum runs as fp8 DoubleRow matmuls with the two kw blocks
    of a pair as the reduction k-tiles; per-batch sigmoids (scale 1/256
    folds the alpha prescale away, and the 1/64 v scale was folded into
    the v cast) write 32-aligned quadrants of packed output tiles, so
    the tail is two residual adds and eight 32KB output DMAs.
  Host: gathers per-core (128, 512) outputs, transposes back.
"""

import math

import numpy as np
import ml_dtypes

import concourse.bass as bass
import concourse.bacc as bacc
import concourse.mybir as mybir
import concourse.tile as tile
import concourse.bass_utils as bass_utils

N_CORES = 8
BS, W_DIM, D = 64, 512, 16
K = W_DIM * D            # 8192 contraction dim
CPC = K // N_CORES       # 1024 output cols per core
WPC = W_DIM // N_CORES   # 64 w positions per core
BPC = BS // N_CORES      # 8 batches per core
NKT = K // 128           # 64 k-tiles
CHUNK = 8                # k-tiles per weight DMA (4 DoubleRow pairs)
NCH = NKT // CHUNK       # 8 weight chunks
WSCALE = 64.0            # host-side weight pre-scale (fp8 subnormal dodge)
QKSCALE = 8.0            # q/k payload post-scale divisor (fp8 range fit)
EASCALE = 256.0          # exp'd-alpha pre-scale (fp8 subnormal dodge)
EXP_SCALE = QKSCALE * QKSCALE / (math.sqrt(K) * WSCALE * WSCALE)

_CACHE: dict = {}


def _build(wbufs: int = 5):
    f8 = mybir.dt.float8e4
    bf16 = mybir.dt.bfloat16
    f32 = mybir.dt.float32
    DR = mybir.MatmulPerfMode.DoubleRow

    nc = bacc.Bacc("TRN2", target_bir_lowering=False, debug=False,
                   num_devices=N_CORES)

    # xt is host-preswizzled to [128 p, 64 kt, 64 b] so the load is one
    # fully contiguous DMA.
    xt_d = nc.dram_tensor("xt", [128, NKT * BS], f8, kind="ExternalInput")
    # weights host-swizzled to [m, p, j, c] -> each 128-row slice is one
    # contiguous 1MB chunk whose rows are 8KB.
    w_d = [nc.dram_tensor(n, [NCH * 128, CHUNK * CPC], f8,
                          kind="ExternalInput")
           for n in ("wq", "wk", "wv")]
    b_d = [nc.dram_tensor(n, [1, CPC], bf16, kind="ExternalInput")
           for n in ("bq", "bk", "bv")]
    # residual x^T, padded to 32-partition quadrants (4 batches per tile,
    # batch q at partitions [32q, 32q+16))
    xtp_d = nc.dram_tensor("xtp", [2 * 128, W_DIM], f32, kind="ExternalInput")
    out_d = nc.dram_tensor("out", [BPC * D, W_DIM], f32, kind="ExternalOutput")

    hwdge = [nc.sync, nc.scalar]          # the two independent HWDGE rings

    with tile.TileContext(nc) as tc:
        with (
            tc.tile_pool(name="constp", bufs=1) as constp,
            tc.tile_pool(name="sbp", bufs=1) as sbp,
            tc.tile_pool(name="dramp", bufs=1, space="DRAM") as dramp,
            tc.tile_pool(name="wpa", bufs=wbufs) as wpa,
            tc.tile_pool(name="wpb", bufs=wbufs) as wpb,
        ):
            xt_sb = constp.tile([128, NKT, BS], f8)
            nc.sync.dma_start(
                xt_sb[:], xt_d[:, :].rearrange("p (kt b) -> p kt b", kt=NKT))
            xtp_sb = constp.tile([128, 2, W_DIM], f32)
            nc.scalar.dma_start(
                xtp_sb[:], xtp_d[:, :].rearrange("(g p) w -> p g w", g=2))
            ones = constp.tile([1, BS], bf16)
            nc.gpsimd.memset(ones[:], 1.0)
            # warm the ScalarE Exp table off the critical path (the
            # table is single-slot, so only one function can be warmed)
            tbl = constp.tile([1, 8], f32)
            nc.scalar.activation(tbl[:], ones[:, 0:8],
                                 mybir.ActivationFunctionType.Exp)
            b_sb = []
            for t in range(3):
                bt = constp.tile([1, CPC], bf16, name=f"bias{t}")
                nc.scalar.dma_start(bt[:], b_d[t][:, :])
                b_sb.append(bt)

            qk_sb = sbp.tile([BS, 2 * CPC], f8, name="qk_sb", tag="qk_sb")
            v_sb = sbp.tile([BS, CPC], f8, name="v_sb", tag="v_sb")
            a2a1_in = dramp.tile([N_CORES, 2, BPC, CPC], f8,
                                 tag="a2a1_in", name="a2a1_in")
            a2a1_out = dramp.tile([N_CORES, 2, BPC, CPC], f8,
                                  tag="a2a1_out", name="a2a1_out")
            a2a2_in = dramp.tile([N_CORES, BPC, CPC], f8,
                                 tag="a2a2_in", name="a2a2_in")
            a2a2_out = dramp.tile([N_CORES, BPC, CPC], f8,
                                  tag="a2a2_out", name="a2a2_out")

            def w_chunk_matmuls(m, psum, wt):
                for jj in range(0, CHUNK, 2):
                    for h in range(2):
                        nc.tensor.matmul(
                            psum[:, h * 512:(h + 1) * 512],
                            xt_sb[:, CHUNK * m + jj:CHUNK * m + jj + 2, :],
                            wt[:, jj:jj + 2, h * 512:(h + 1) * 512],
                            start=(m == 0 and jj == 0), stop=False,
                            perf_mode=DR)

            def bias_matmuls(t, psum):
                for h in range(2):
                    nc.tensor.matmul(
                        psum[:, h * 512:(h + 1) * 512],
                        ones[:],
                        b_sb[t][:, h * 512:(h + 1) * 512],
                        start=False, stop=True)

            # ---- phase QK: q,k projection ----
            with tc.tile_pool(name="qkps", bufs=1, space="PSUM") as qkps:
                psA = [qkps.tile([BS, CPC], f32, name=f"ps{t}")
                       for t in range(2)]
                for m in range(NCH):
                    for t in range(2):
                        wt = wpa.tile([128, CHUNK, CPC], f8,
                                      tag=f"w{t}", name=f"wt{t}")
                        hwdge[(m * 2 + t) % 2].dma_start(
                            wt[:],
                            w_d[t][128 * m:128 * (m + 1), :].rearrange(
                                "p (j c) -> p j c", j=CHUNK))
                        w_chunk_matmuls(m, psA[t], wt)
                for t in range(2):
                    bias_matmuls(t, psA[t])
                    # permute cols (w d) -> (d w) while leaving PSUM;
                    # scale into fp8 payload range
                    nc.vector.tensor_scalar_mul(
                        qk_sb[:, t * CPC:(t + 1) * CPC].rearrange(
                            "b (d w) -> b d w", w=WPC),
                        psA[t].rearrange("b (w d) -> b d w", d=D),
                        1.0 / QKSCALE)

            for t in range(2):
                nc.gpsimd.dma_start(
                    a2a1_in[:, t, :, :],
                    qk_sb[:, t * CPC:(t + 1) * CPC])
            # hold v streaming back until the staging data lands so the
            # A2A1 trigger isn't starved by DMA-engine congestion
            tc.strict_bb_all_engine_barrier()
            nc.gpsimd.collective_compute(
                "AllToAll", mybir.AluOpType.bypass,
                replica_groups=[list(range(N_CORES))],
                ins=[a2a1_in.opt()], outs=[a2a1_out.opt()])

            # ---- attention pools open early so the q/k gathers sit
            # ahead of the v staging + A2A2 trigger in the gpsimd FIFO ----
            with (
                tc.tile_pool(name="attps", bufs=1, space="PSUM") as attps,
                tc.tile_pool(name="attp", bufs=3) as attp,
                tc.tile_pool(name="keepp", bufs=1) as keepp,
            ):
                # dens + q/k gathers up-front: memsets run under the
                # collective, gathers fire the moment it completes
                qkT_tiles = []
                den_tiles = []
                for b in range(BPC):
                    den = keepp.tile([128, 4], f32, tag=f"den{b}",
                                     name=f"den{b}")
                    nc.gpsimd.memset(den[:], 0.0)
                    den_tiles.append(den)
                for b in range(BPC):
                    qkT = keepp.tile([D, 2, N_CORES, WPC], f8,
                                     tag=f"qkT{b}", name=f"qkT{b}")
                    for t in range(2):
                        nc.gpsimd.dma_start(
                            qkT[:, t, :, :],
                            a2a1_out[:, t, b, :].rearrange(
                                "i (d w) -> d i w", d=D))
                    qkT_tiles.append(qkT)

                # ---- phase V: v projection, overlapped with the q/k
                # AllToAll and attention part A ----
                with tc.tile_pool(name="vps", bufs=1, space="PSUM") as vps:
                    psV = vps.tile([BS, CPC], f32, name="psv")
                    for m in range(NCH):
                        wt = wpb.tile([128, CHUNK, CPC], f8, tag="w2",
                                      name="wt2")
                        hwdge[m % 2].dma_start(
                            wt[:],
                            w_d[2][128 * m:128 * (m + 1), :].rearrange(
                                "p (j c) -> p j c", j=CHUNK))
                        w_chunk_matmuls(m, psV, wt)
                    bias_matmuls(2, psV)
                    # fold the 1/64 weight prescale into the fp8 v payload
                    nc.vector.tensor_scalar_mul(v_sb[:], psV[:],
                                                1.0 / WSCALE)

                nc.gpsimd.dma_start(
                    a2a2_in[:, :, :].rearrange("j b c -> j b c"),
                    v_sb[:, :])
                nc.gpsimd.collective_compute(
                    "AllToAll", mybir.AluOpType.bypass,
                    replica_groups=[list(range(N_CORES))],
                    ins=[a2a2_in.opt()], outs=[a2a2_out.opt()])
                a2a_v = a2a2_out.rearrange("(kw h) b c -> kw h b c", h=2)

                # v tiles for all batches: resident long before part B
                vt_tiles = []
                for b in range(BPC):
                    vt = keepp.tile([128, 4, D], f8, tag=f"vt{b}",
                                    name=f"vt{b}")
                    for half in range(2):
                        hwdge[half].dma_start(
                            vt[64 * half:64 * half + 64, :, :],
                            a2a_v[:, half, b, :].rearrange(
                                "i (w d) -> w i d", d=D))
                    vt_tiles.append(vt)

                sg_tiles = [keepp.tile([128, W_DIM], f32, tag=f"sg{g}",
                                       name=f"sg{g}") for g in range(2)]
                eas_tiles = {}
                for b in range(BPC):
                    qkT = qkT_tiles[b]
                    den = den_tiles[b]
                    ea_tiles = []
                    for j in range(2):
                        aT2 = attps.tile([128, 2, 512], f32, tag="aT2",
                                         name="aT2", bufs=2)
                        for h in range(2):
                            kw = 2 * j + h
                            nc.tensor.matmul(
                                aT2[:, h, :], qkT[:, 1, 2 * kw:2 * kw + 2, :],
                                qkT[:, 0, :, :], start=True, stop=True)
                            ea = attp.tile([128, 512], bf16, tag=f"ea{kw}",
                                           name="ea")
                            nc.scalar.activation(
                                ea[:], aT2[:, h, :],
                                mybir.ActivationFunctionType.Exp,
                                scale=EXP_SCALE, accum_out=den[:, kw:kw + 1])
                            ea_tiles.append(ea)
                    rec = attp.tile([128, 4], f32, tag="rec", name="rec")
                    nc.vector.reciprocal(rec[:], den[:])
                    eas_pair = []
                    for j in range(2):
                        # fold softmax denominator; x256 keeps the near-
                        # uniform softmax weights in fp8 normal range
                        eas = keepp.tile([128, 2, 512], f8,
                                         tag=f"eas{b}_{j}",
                                         name=f"eas{b}_{j}")
                        for h in range(2):
                            kw = 2 * j + h
                            nc.vector.tensor_scalar(
                                eas[:, h, :], ea_tiles[kw][:],
                                rec[:, kw:kw + 1], EASCALE,
                                op0=mybir.AluOpType.mult,
                                op1=mybir.AluOpType.mult)
                        eas_pair.append(eas)
                    eas_tiles[b] = eas_pair
                # part B as a separate loop: keeps the in-order scalar
                # queue as [all exps][all sigmoids] with no mid-stalls
                for b in range(BPC):
                    rT = attps.tile([D, W_DIM], f32, tag="rT", name="rT",
                                    bufs=2)
                    for j in range(2):
                        # fp8 DoubleRow: the two kw blocks of a pair ride
                        # as the two reduction k-tiles
                        nc.tensor.matmul(
                            rT[:], vt_tiles[b][:, 2 * j:2 * j + 2, :],
                            eas_tiles[b][j][:],
                            start=(j == 0), stop=(j == 1),
                            perf_mode=DR)
                    # per-batch sigmoid straight from PSUM into the
                    # 32-aligned quadrant of the packed output tile
                    nc.scalar.activation(
                        sg_tiles[b // 4][32 * (b % 4):32 * (b % 4) + D, :],
                        rT[:], mybir.ActivationFunctionType.Sigmoid,
                        scale=1.0 / EASCALE)
                for g in range(2):
                    oo = keepp.tile([128, W_DIM], f32, tag=f"oo{g}",
                                    name=f"oo{g}")
                    nc.vector.tensor_add(oo[:], sg_tiles[g][:],
                                         xtp_sb[:, g, :])
                    for b4 in range(4):
                        b = 4 * g + b4
                        hwdge[b % 2].dma_start(
                            out_d[D * b:D * (b + 1), :],
                            oo[32 * b4:32 * b4 + D, :])

    nc.compile()
    return nc


def _prep_in_maps(x_in, Wq, bq, Wk, bk, Wv, bv):
    f8 = ml_dtypes.float8_e4m3
    bf16 = ml_dtypes.bfloat16
    x_flat = np.ascontiguousarray(np.asarray(x_in, np.float32).reshape(BS, K))
    # swizzled x^T: [128 p, kt, b] contiguous
    xt = np.ascontiguousarray(
        x_flat.T.reshape(NKT, 128, BS).transpose(1, 0, 2)
    ).reshape(128, NKT * BS).astype(f8)
    # W^T scaled by 64, swizzled to [m, p, j, c] per core slice
    ws = [np.ascontiguousarray(np.asarray(W, np.float32).T) * WSCALE
          for W in (Wq, Wk, Wv)]
    bs = [(np.asarray(b, np.float32) * WSCALE).reshape(1, K).astype(bf16)
          for b in (bq, bk, bv)]
    xtp = np.ascontiguousarray(
        np.asarray(x_in, np.float32).transpose(0, 2, 1))       # (BS, D, W)

    in_maps = []
    for c in range(N_CORES):
        cs = slice(CPC * c, CPC * (c + 1))
        m = {"xt": xt}
        for nm, w in zip(("wq", "wk", "wv"), ws):
            m[nm] = np.ascontiguousarray(
                w[:, cs].reshape(NCH, CHUNK, 128, CPC).transpose(0, 2, 1, 3)
            ).reshape(NCH * 128, CHUNK * CPC).astype(f8)
        for nm, b in zip(("bq", "bk", "bv"), bs):
            m[nm] = np.ascontiguousarray(b[:, cs])
        xp = np.zeros((2, 4, 32, W_DIM), np.float32)
        xp[:, :, :D, :] = xtp[BPC * c:BPC * (c + 1)].reshape(2, 4, D, W_DIM)
        m["xtp"] = xp.reshape(2 * 128, W_DIM)
        in_maps.append(m)
    return in_maps


def _assemble(results):
    out = np.empty((BS, W_DIM, D), np.float32)
    for c in range(N_CORES):
        o = results[c]["out"].reshape(BPC, D, W_DIM)
        out[BPC * c:BPC * (c + 1)] = o.transpose(0, 2, 1)
    return out


def get_nc():
    if "nc" not in _CACHE:
        _CACHE["nc"] = _build()
    return _CACHE["nc"]


def kernel(x_in, Wq, bq, Wk, bk, Wv, bv):
    nc = get_nc()
    in_maps = _prep_in_maps(x_in, Wq, bq, Wk, bk, Wv, bv)
    res = bass_utils.run_bass_kernel_spmd(
        nc, in_maps, core_ids=list(range(N_CORES)))
    return _assemble(res.results)
